# revision 56
# baseline (speedup 1.0000x reference)
"""Trainium2 Bass kernel for nn_Attention_26079041421696.

Full-volume single-head-per-core attention (8 heads -> 8 NeuronCores,
tensor-parallel on the head axis per the sharding hint).

Math per core h (n=4096 tokens, C=256 channels, dh=64):
    q = x @ (0.25*wq_h), k = x @ (0.25*wk_h), v = x @ wv_h
    simT[j, i] = q_i . k_j = 0.0625 * raw_sim     (scale pre-folded)
    p = exp(2 * simT)          = exp(0.125 * raw_sim)
    oT[d, i]  = sum_j v[j, d] p[j, i]             (unnormalized)
    den[i]    = sum_j p[j, i]  (ones-column appended to v -> row 64)
Host epilogue: out = sum_h w_out_h.T @ (oT_h / den_h) + b_out
(the 1x1 output conv runs on host in fp32).

Key design points (from neuron-profile traces of prior versions):
  - EVERY attention matmul is a K=64 row-group op, so the PE array never
    switches between row-tiled and full-array configs. The v2 profile
    showed each sim<->av transition exposing a ~160ns pipeline drain
    (full-array av must wait for the row-tiled sim pair to drain and
    vice versa); with a uniform config the stream pipelines at the
    ~213ns N=512 issue rate.
      - sim: chunk c -> row group 64*(c%2) as before (qT/kT duplicated
        into both partition halves).
      - av: the K=128 token contraction of each chunk is SPLIT into two
        concurrent K=64 matmuls: tokens 0:64 (rg0) -> ot_a, tokens
        64:128 (rg1) -> ot_b. Separate accumulator banks make the
        concurrent accumulation race-free; one VectorE tensor_add per
        pass merges them during the drain it had to do anyway.
  - qk projection computes q and k concurrently via PE column groups
    (psqk[0:64]=qT, psqk[64:128]=kT), two i-tiles per weight set.
  - exp is per-chunk [128,512]: ScalarE Exp (scale=2.0) or a CUSTOM
    FUSED VectorE op CSQ_EXP_ANT computing ((x+H)^2 + K/A)^2 * A^2 in a
    single DVE instruction (5 chained ALU slices) -- same quadratic
    exp(2x) fit the baseline evaluated in 4 separate DVE ops. The
    chunk->engine split is tuned so both engines carry ~95us, under the
    TensorE critical path (~98us union busy).
  - x is DMA'd in 512-token chunks so qk starts ~1us into the kernel;
    the exp table set is preloaded during the input DMA.
"""

import numpy as np
import ml_dtypes

HEADS = 8
DH = 64
N_TOK = 4096
C_IN = 256
SCALE = DH ** -0.5
N_CORES = 8

CSQ_H = 1.03195340625305
CSQ_A = 0.4920321333500102
CSQ_K = 0.47663991970600067

# of each 64-chunk window, which chunks' exp runs on VectorE (custom op)
# vs ScalarE (Exp activation). 31/64 = 124 of 256 units on DVE.
DVE_NUM, DVE_DEN = 31, 64

AV_LAG = 8  # chunks the av matmuls trail the sims by

_CACHE = {}


def register_csq_exp():
    """Register the CSQ_EXP_ANT custom DVE op (idempotent): one VectorE
    instruction computing ((x + s0)^2 + s1)^2 * imm2."""
    from concourse import dve_ops
    from concourse.dve_spec import C0, C1, C2, Spec, Src0, lower, sq
    from concourse.dve_uop import DveOpSpec

    for o in dve_ops.OPS:
        if o.name == "CSQ_EXP_ANT":
            return o

    spec = Spec(
        body=sq(sq(Src0 + C0) + C1) * C2,
        reference=lambda in0, in1, s0, s1, imm2: (
            (((in0.astype(np.float32) + s0) ** 2 + s1) ** 2) * imm2
        ),
    )
    row = max(dve_ops._SUB_OPCODE_FOR_NAME.values()) + 1
    assert row < 0x20
    dve_ops._SUB_OPCODE_FOR_NAME["CSQ_EXP_ANT"] = row
    shas = {}
    for ver in ("v3", "v4"):
        s = DveOpSpec(name="CSQ_EXP_ANT", opcode=row, uops=lower(spec, ver=ver),
                      rd1_en=False)
        shas[ver] = s.sha(ver)
    op = dve_ops.DveOp("CSQ_EXP_ANT", spec, subdim=False, uops_sha=shas)
    dve_ops.OPS.append(op)
    dve_ops.CUSTOM_DVE_SPECS["CSQ_EXP_ANT"] = spec
    return op


def build_nc():
    """Build + compile the per-core Bass/Tile graph (same program on all 8
    cores; only the input data differs per core)."""
    import concourse.bacc as bacc
    import concourse.mybir as mybir
    from concourse import tile

    bf16 = mybir.dt.bfloat16
    f16 = mybir.dt.float16
    f32 = mybir.dt.float32
    Exp = mybir.ActivationFunctionType.Exp
    csq = register_csq_exp()

    nc = bacc.Bacc("TRN2", target_bir_lowering=False, debug=False)

    xT_d = nc.dram_tensor("xT", [C_IN, N_TOK], bf16, kind="ExternalInput")
    wqkv_d = nc.dram_tensor("wqkv", [128, 384], bf16, kind="ExternalInput")
    oT_d = nc.dram_tensor("oT", [DH + 1, N_TOK], f16, kind="ExternalOutput")

    with tile.TileContext(nc) as tc:
        with (
            tc.tile_pool(name="cpool", bufs=1) as cpool,
            tc.tile_pool(name="spool", bufs=2) as spool,
            tc.tile_pool(name="pspool", bufs=2, space="PSUM") as pspool,
        ):
            # ---- persistent SBUF tiles -------------------------------
            x0 = cpool.tile([128, N_TOK], bf16, tag="x0")
            x1 = cpool.tile([128, N_TOK], bf16, tag="x1")
            wqkv = cpool.tile([128, 384], bf16, tag="wqkv")
            # A: rows 0:64 = qT, rows 64:128 = kT (psqk bank layout);
            # B: rows 0:64 = kT, rows 64:128 = qT (the swap, via DMA dups).
            # Row group rg reads q from (A if rg==0 else B)[rg] and k from
            # the other -- so one FULL-partition copy drains each psqk bank
            # (half the copy instructions of separate qT/kT tiles).
            A_sb = cpool.tile([128, N_TOK], bf16, tag="qq")
            B_sb = cpool.tile([128, N_TOK], bf16, tag="kk")
            v_sb = cpool.tile([128, 32 * 65], f16, tag="v")
            warm = cpool.tile([128, 2], f16, tag="warm")

            # wqkv rides the scalar HWDGE queue so it transfers in
            # PARALLEL with x0[0:512] (sync queue) -- the two first-matmul
            # gates previously serialized on one queue (~1.5us)
            nc.scalar.dma_start(wqkv[:], wqkv_d[:])
            # x channel halves ride the TWO HWDGE queues (SP + Activation)
            # so the ~600ns per-DMA issue cost parallelizes; only the first
            # 1024 tokens come before qk pair 0 is emitted, so pair 0's
            # qqT/kkT duplicate DMAs (which gate the first sims) issue
            # ahead of the remaining x traffic
            for a, b in ((0, 512), (512, 1024)):
                nc.sync.dma_start(x0[:, a:b], xT_d[0:128, a:b])
                nc.scalar.dma_start(x1[:, a:b], xT_d[128:256, a:b])

            # ---- P1: q, k (transposed, [64, 4096]) -------------------
            # psqk bank: rows 0:64 = qT i-tile (col group 0), rows 64:128 =
            # kT i-tile (col group 64) -- q and k matmuls run concurrently
            # in disjoint PE column groups. Two i-tiles share each weight
            # set so the mid-accumulation weight switch happens once per
            # pair instead of once per tile.
            def emit_qk_pair(it0):
                pss = []
                for it in (it0, it0 + 1):
                    pss.append(pspool.tile([128, 512], f32, tag="acc", bufs=4,
                                           name=f"psqk{it}"))
                for ch, xx in ((0, x0), (1, x1)):
                    for k, it in enumerate((it0, it0 + 1)):
                        sl = slice(it * 512, (it + 1) * 512)
                        nc.tensor.matmul(pss[k][0:64, :],
                                         wqkv[:, ch * 64 : ch * 64 + 64],
                                         xx[:, sl], start=(ch == 0), stop=(ch == 1))
                        nc.tensor.matmul(pss[k][64:128, :],
                                         wqkv[:, 128 + ch * 64 : 192 + ch * 64],
                                         xx[:, sl], start=(ch == 0), stop=(ch == 1))
                for k, it in enumerate((it0, it0 + 1)):
                    sl = slice(it * 512, (it + 1) * 512)
                    if it0 == 0:
                        # first pair: q and k land in partitions 0:64
                        # directly (separate engine copies) so the early
                        # rg0-only sims never wait on the duplicate DMAs
                        nc.scalar.copy(A_sb[0:64, sl], pss[k][0:64, :])
                        nc.vector.tensor_copy(B_sb[0:64, sl], pss[k][64:128, :])
                        nc.sync.dma_start(A_sb[64:128, sl], B_sb[0:64, sl])
                        nc.sync.dma_start(B_sb[64:128, sl], A_sb[0:64, sl])
                    else:
                        # one full-partition copy drains the whole bank
                        if it % 2 == 0:
                            nc.scalar.copy(A_sb[:, sl], pss[k][:])
                        else:
                            nc.vector.tensor_copy(A_sb[:, sl], pss[k][:])
                        nc.sync.dma_start(B_sb[0:64, sl], A_sb[64:128, sl])
                        nc.sync.dma_start(B_sb[64:128, sl], A_sb[0:64, sl])

            # pair (0,1) up front (covers kkT chunks 0-7 / qqT pass 0);
            # pairs (2,3),(4,5),(6,7) weave into the early attention groups
            # below, so the PE stream never stalls on a late x chunk while
            # ready sim/av work sits behind it
            emit_qk_pair(0)
            # v_sb ones-columns -- emitted AFTER qk pair 0 so the kkT
            # copies that gate the first sims sit ahead in the Vector queue
            nc.vector.memset(v_sb[:], 1.0)
            # preload the exp table set (2.7us ACT_TABLE_LOAD) now: after
            # the first x1 chunk's DMA issue (it would delay it at the
            # queue head) but before the tail x1 chunks (needed only ~8us
            # in, they can afford to queue behind it)
            nc.scalar.activation(warm[:, 0:1], warm[:, 1:2], Exp, scale=0.0)
            for a, b in ((1024, 2048), (2048, 3072), (3072, 4096)):
                nc.sync.dma_start(x0[:, a:b], xT_d[0:128, a:b])
                nc.scalar.dma_start(x1[:, a:b], xT_d[128:256, a:b])

            # ---- P2: attention ---------------------------------------
            # 8 passes, one 512-wide i-tile each; 32 j-chunks per pass.
            # Flat schedule over 256 global chunks (pass gh = gc//32): avs
            # trail sims by AV_LAG chunks ACROSS pass boundaries, so the
            # end-of-pass av flush never stalls the in-order PE stream on
            # the last chunks' exp latency. Each pass's accumulator pair
            # drains (merge + DMA) right after its last av, ~4 groups into
            # the next pass.
            p_tiles = {}
            ots = {}  # gh -> (ot_a, ot_b)

            def emit_av(gc, halves=(0, 1)):
                # The K=128 token contraction of chunk gc splits into two
                # CONCURRENT K=64 row-group matmuls into separate
                # accumulator banks (same-bank accumulation across row
                # groups faults at runtime); the pass drain merges them.
                gh, c = gc // 32, gc % 32
                if gh not in ots:
                    ots[gh] = (
                        pspool.tile([DH + 1, 512], f32, tag="acc",
                                    bufs=4, name=f"ota{gh}"),
                        pspool.tile([DH + 1, 512], f32, tag="acc",
                                    bufs=4, name=f"otb{gh}"),
                    )
                p_sb = p_tiles[gc]
                for hi in halves:
                    rs = slice(64 * hi, 64 * hi + 64)
                    nc.tensor.matmul(ots[gh][hi][:],
                                     v_sb[rs, c * 65 : c * 65 + 65],
                                     p_sb[rs, :], start=(c == 0), stop=(c == 31))
                if halves[-1] == 1:
                    del p_tiles[gc]
                    if c == 31:
                        emit_epilogue(gh)

            def emit_epilogue(gh):
                ota, otb = ots.pop(gh)
                oa_sb = spool.tile([DH + 1, 512], f32, tag="oasb", bufs=2,
                                   name=f"oa{gh}")
                oT_sb = spool.tile([DH + 1, 512], f16, tag="otsb", bufs=2,
                                   name=f"osb{gh}")
                # merge rg0/rg1 accumulators (row 64 = denominator); the
                # last pass pipelines copy->add->DMA in column halves since
                # it sits on the serial tail
                cols = ((0, 256), (256, 512)) if gh == 7 else ((0, 512),)
                for a, b in cols:
                    nc.scalar.copy(oa_sb[:, a:b], ota[:, a:b])
                for a, b in cols:
                    nc.vector.tensor_add(oT_sb[:, a:b], oa_sb[:, a:b],
                                         otb[:, a:b])
                    nc.sync.dma_start(
                        oT_d[:, gh * 512 + a : gh * 512 + b], oT_sb[:, a:b]
                    )

            av_done = 0

            def emit_avs_until(limit):
                nonlocal av_done
                while av_done < min(limit, 256):
                    emit_av(av_done)
                    av_done += 1

            QK_AT = {2: 2, 5: 4, 8: 6}
            PSV_AT = {1: 0, 3: 1, 4: 2, 6: 3}
            for g in range(128):  # global 2-chunk groups
                if g in QK_AT:
                    emit_qk_pair(QK_AT[g])
                if g in PSV_AT:
                    # v = x @ wv, 8 token-chunks per PSUM bank, woven
                    # into the first pass
                    blk = PSV_AT[g]
                    psv = pspool.tile([128, 512], f32, tag="acc", bufs=4,
                                      name=f"psv{blk}")
                    for cc in range(8):
                        tck = blk * 8 + cc
                        slt = slice(tck * 128, (tck + 1) * 128)
                        nc.tensor.matmul(psv[:, cc * DH : (cc + 1) * DH],
                                         x0[:, slt], wqkv[:, 256:320],
                                         start=True, stop=False)
                        nc.tensor.matmul(psv[:, cc * DH : (cc + 1) * DH],
                                         x1[:, slt], wqkv[:, 320:384],
                                         start=False, stop=True)
                    vdst = v_sb[:, blk * 520 : (blk + 1) * 520]
                    vdst = vdst.rearrange("p (a b) -> p a b", b=65)[:, :, 0:DH]
                    nc.vector.tensor_copy(
                        vdst, psv[:].rearrange("p (a b) -> p a b", b=DH)
                    )
                # avs first: they trail by AV_LAG chunks so their inputs are
                # long ready; putting them ahead of the sims in the in-order
                # PE queue absorbs the sims' exp-completion waits (~400ns
                # median, measured) under av streaming.
                emit_avs_until(2 * g + 2 - AV_LAG)
                for s in range(2):
                    gc = 2 * g + s
                    gh, c = gc // 32, gc % 32
                    # chunks 0-7: row group 0 only -- the partition-64:128
                    # duplicates of qqT/kkT haven't landed yet and these
                    # sims are input-DMA-shadowed regardless
                    rg = 0 if gc < 8 else 64 * (c % 2)
                    k_src = B_sb if rg == 0 else A_sb
                    q_src = A_sb if rg == 0 else B_sb
                    pst = pspool.tile([128, 512], f32, tag="sim", bufs=4,
                                      name=f"pst{gc}")
                    nc.tensor.matmul(
                        pst[:],
                        k_src[rg : rg + 64, c * 128 : (c + 1) * 128],
                        q_src[rg : rg + 64, gh * 512 : (gh + 1) * 512],
                        start=True, stop=True,
                    )
                    p_sb = spool.tile([128, 512], f16, tag="p", bufs=20,
                                      name=f"p{gc}")
                    if (gc * DVE_NUM) % DVE_DEN < DVE_NUM:
                        nc.vector._custom_dve(
                            csq, out=p_sb[:], in0=pst[:],
                            s0=CSQ_H, s1=CSQ_K / CSQ_A,
                            imm2=CSQ_A * CSQ_A,
                        )
                    else:
                        nc.scalar.activation(p_sb[:], pst[:], Exp, scale=2.0)
                    p_tiles[gc] = p_sb
            # final flush: all rg0 halves first, then rg1 -- ot_a completes
            # early so the epilogue's scalar copy overlaps the rg1 stream
            rest = range(av_done, 256)
            for gc in rest:
                emit_av(gc, halves=(0,))
            for gc in rest:
                emit_av(gc, halves=(1,))
            av_done = 256

    nc.compile()
    return nc


def make_in_maps(x, w_qkv):
    """Host-side shard prep: transpose + bf16-cast x (shared), slice the
    qkv weights per head. q,k weights carry a 0.25 factor each so the sim
    matmul directly yields 0.0625*raw_sim (the exp argument / 2)."""
    bf = ml_dtypes.bfloat16
    xf = np.asarray(x, np.float32).reshape(N_TOK, C_IN)
    xT = np.ascontiguousarray(xf.T).astype(bf)
    w_qkv = np.asarray(w_qkv, np.float32)
    in_maps = []
    for h in range(HEADS):
        wq = w_qkv[:, h * DH : (h + 1) * DH] * 0.25
        wk = w_qkv[:, 512 + h * DH : 512 + (h + 1) * DH] * 0.25
        wv = w_qkv[:, 1024 + h * DH : 1024 + (h + 1) * DH]
        wqkv_np = np.concatenate(
            [wq[:128], wq[128:], wk[:128], wk[128:], wv[:128], wv[128:]], axis=1
        ).astype(bf)
        in_maps.append({"xT": xT, "wqkv": wqkv_np})
    return in_maps


def postprocess(results, w_out, b_out):
    """Combine per-core partials: normalize, project (1x1 out-conv on host
    in fp32), sum heads, add bias."""
    w_out = np.asarray(w_out, np.float32)
    o_all = np.empty((HEADS * DH, N_TOK), np.float32)
    for h in range(HEADS):
        oT = np.asarray(results[h]["oT"], dtype=np.float32)
        o_all[h * DH : (h + 1) * DH] = oT[0:DH] / oT[DH][None, :]
    out = o_all.T @ w_out + np.asarray(b_out, np.float32)[None, :]
    return out.astype(np.float32).reshape(1, 8, 16, 32, C_IN)


def kernel(x, w_qkv, w_out, b_out):
    from concourse.bass_utils import run_bass_kernel_spmd

    nc = _CACHE.get("nc")
    if nc is None:
        nc = build_nc()
        _CACHE["nc"] = nc
    in_maps = make_in_maps(x, w_qkv)
    res = run_bass_kernel_spmd(nc, in_maps, core_ids=list(range(N_CORES)))
    return postprocess(res.results, w_out, b_out)
